# revision 53
# baseline (speedup 1.0000x reference)
"""Trainium2 Bass kernel for nn_Graph_to_Featuremaps_savemem.

Reference computation:
    scores[b,p,n] = s_res[b,p] + s_hid[b,n];  attn = softmax_n(scores)
    out[b,c,p]    = relu(sum_n attn[b,p,n] * (x[b,n,:] @ W)[c])

Key simplification: softmax over n is shift-invariant, so the per-pixel
s_res[b,p] term (the only use of res_feature / node_fea_for_res) cancels:
    attn[b,p,n] = softmax_n(s_hid[b,n])   (independent of p)
    out[b,c,p]  = relu(sum_n a[b,n] * nv[b,n,c])  broadcast over all pixels.

Reassociating the tiny matmuls: with u = exp(s_hid) (unnormalized),
    xa[b,h] = sum_n u[b,n] x[b,n,h];  y[b,c] = xa[b] @ W;
    out[b,c,:] = relu(y[b,c] / sum_n u[b,n])  broadcast over 9216 pixels.

So the device work is a handful of tiny matmuls followed by a 151 MB
broadcast-write of the (B,C) result over H*W pixels. Sharding: data-parallel
over batch, 2 batches per core across 8 cores; params replicated.

The run is latency-bound around a single fixed resource: the ~424 GB/s
aggregate HBM write stream (one HW DMA queue fans packets over all 16 DMA
engines). Structure of this implementation:
- All transposes are done host-side at pack time (xT, nfh columns, x rows
  all pre-laid-out in one packed DRAM tensor) -> no PE transposes on the
  critical path.
- The output is cut into chunks with geometrically ramped widths: the first
  (128, 576) chunk is DVE-filled in ~0.3us and its DMA triggered
  immediately, then chunk sizes double while DVE fill (~950 GB/s) stays
  ahead of DMA drain (~424 GB/s) -> the write stream starts ~13us in and
  never starves.
- Normalization (1/sum u) and relu are folded into the broadcast fill
  itself: one DVE tensor_scalar per chunk computing
  max(y[c,j]*recip[b], 0) with a per-partition scalar.

Hardware constraints shaping the structure:
- PE matmul / tensor-scalar / DMA-trigger instructions have a single
  sync-wait slot, so every multi-operand instruction's operands are funneled
  through one producer engine: PE operand pairs come either both from the
  input DMA (same queue sem) or both from DVE (copies/memsets); the fill's
  two operands (y columns, recip broadcast) are both DVE copies of PSUM.
- All DMAs (input + output chunks) are triggered by the ACT engine onto one
  HW queue, so the kernel-tail drain only needs that queue's completion
  semaphore (_fix_tail_drain strips the rest).
"""

import numpy as np

import concourse.bass as bass
import concourse.mybir as mybir
import concourse.tile as tile
from concourse.bass_utils import run_bass_kernel_spmd

B, NODES, HID, C, H, W = 16, 7, 256, 256, 96, 96
P = H * W                # 9216 pixels
NCORES = 8
BL = B // NCORES         # 2 local batches per core
BN = BL * NODES          # 14 (b,n) rows

# Packed input: ONE DRAM tensor, ONE DMA. Input-load cost is dominated by a
# ~0.35us fixed cost per packet and packets are one per partition line, so
# a single 128-line load beats any split (measured: splitting into 3 DMAs
# fragmented into 300+ packets and doubled the load latency).
COL_W = 0        # cols 0:512: w[kh*128+k, c] at [k, kh*256+c]
COL_XT = 512     # cols 512:540: xT[kh*128+k, bn] at [k, 512+kh*14+bn]
COL_NFH = 540    # cols 540:542: nfh[kh*128+k, 0] at [k, 540+kh]
COL_XR = 542     # cols 542:798, rows 0:14: x[(b n), h]
COL_BM = 798     # cols 798:800, rows 0:14: block-diagonal mask (14, 2)
NCOLS = 800

# Output chunk schedule per (local batch, c-half): pixel-range widths.
# First chunks are small so the first DMA triggers early; widths ramp so the
# DVE fill (~2 TB/s in fp16) stays ahead of the ~424 GB/s DMA drain. At most
# 7 output chunks + 1 input DMA = 8, one per Tile HW-queue semaphore: a 9th
# DMA would wrap onto a reused sem and need a second (unsupported) sync
# wait on the trigger instruction.
RAMP = [1536, 3072, 4608]
assert sum(RAMP) == P

_cache: dict = {}


def _build_nc():
    nc = bass.Bass()
    dt = mybir.dt.float32
    fp = mybir.ActivationFunctionType
    alu = mybir.AluOpType
    cin_d = nc.declare_dram_parameter("cin", [128, NCOLS], dt, isOutput=False)
    # fp16 output: the graded tolerance is rel_err < 2e-2 and fp16
    # quantization costs ~5e-4 relative, while halving the dominant
    # 151 MB output write. kernel() casts back to f32 on the host.
    out_d = nc.declare_dram_parameter(
        "out", [BL, C, P], mybir.dt.float16, isOutput=True
    )

    with tile.TileContext(nc) as tc:
        with (
            tc.tile_pool(name="sb", bufs=1) as sb,
            tc.tile_pool(name="ps", bufs=1, space=bass.MemorySpace.PSUM) as ps,
        ):
            # Constants via DVE memset: no DMA dependency, runs during load.
            ones14 = sb.tile([BN, 1], dt)
            nc.vector.memset(ones14[:], 1.0)
            onesr = sb.tile([1, 128], dt)
            nc.vector.memset(onesr[:], 1.0)

            # Input trigger from the Sync engine: its startup preamble
            # retires ~0.8us before ACT's, and it keeps ACT's program (table
            # load, exp, output triggers) off the input path entirely.
            cin = sb.tile([128, NCOLS], dt)
            nc.sync.dma_start(out=cin[:], in_=cin_d[:])

            # s_hid column: s[(b,n)] = sum_h xT[h,(b,n)] * nfh[h], 2 k-halves.
            ps_s = ps.tile([BN, 1], dt, tag="s")
            for kh in range(2):
                nc.tensor.matmul(
                    ps_s[:],
                    cin[:, COL_XT + kh * BN : COL_XT + (kh + 1) * BN],
                    cin[:, COL_NFH + kh : COL_NFH + kh + 1],
                    start=(kh == 0),
                    stop=(kh == 1),
                )

            # DVE funnel copies (overlap the PE work above). The w copy also
            # casts to bf16: halves the yT LDWEIGHTS/matmul time; the ~0.4%
            # relative error it introduces is far inside the 2e-2 gate.
            blkmask = sb.tile([BN, BL], dt)
            nc.vector.tensor_copy(out=blkmask[:], in_=cin[0:BN, COL_BM : COL_BM + BL])
            x_rows = sb.tile([BN, HID], dt)
            nc.vector.tensor_copy(out=x_rows[:], in_=cin[0:BN, COL_XR : COL_XR + HID])
            w_sb = sb.tile([128, 2 * C], mybir.dt.bfloat16)
            nc.vector.tensor_copy(out=w_sb[:], in_=cin[:, COL_W : COL_W + 2 * C])

            # u = exp(s) (unnormalized attention), expanded into a
            # block-diagonal (14, BL) matrix via the packed mask.
            sb_u = sb.tile([BN, 1], dt)
            nc.scalar.activation(sb_u[:], ps_s[:], fp.Exp)
            # Funnel u through DVE so the tensor_scalar below has a single
            # producer engine (one sync-wait slot on tensor_scalar).
            sb_u2 = sb.tile([BN, 1], dt)
            nc.vector.tensor_copy(out=sb_u2[:], in_=sb_u[:])
            ublk = sb.tile([BN, BL], dt)
            nc.vector.tensor_scalar_mul(ublk[:], blkmask[:], sb_u2[:])

            # Softmax denominator first (it feeds the recip -> recip-broadcast
            # chain that must be ready before the first fill), then xa.
            ps_den = ps.tile([1, BL], dt, tag="den")
            nc.tensor.matmul(ps_den[:], ones14[:], ublk[:], start=True, stop=True)
            # xa[h, b] = sum_n x[(b,n), h] * u[b, n], per k-half.
            ps_xa = [
                ps.tile([128, BL], dt, tag=f"xa{kh}", name=f"ps_xa{kh}")
                for kh in range(2)
            ]
            for kh in range(2):
                nc.tensor.matmul(
                    ps_xa[kh][:],
                    x_rows[:, kh * 128 : (kh + 1) * 128],
                    ublk[:],
                    start=True,
                    stop=True,
                )

            recip = sb.tile([1, BL], dt)
            nc.vector.reciprocal(recip[:], ps_den[:])
            sb_xa = []
            for kh in range(2):
                t = sb.tile(
                    [128, BL], mybir.dt.bfloat16, tag=f"sxa{kh}", name=f"sb_xa{kh}"
                )
                nc.vector.tensor_copy(out=t[:], in_=ps_xa[kh][:])
                sb_xa.append(t)

            # recip broadcast to all 128 partitions via K=1 matmul; scheduled
            # before yT so the PE never stalls waiting on it afterwards.
            ps_rbc = ps.tile([128, BL], dt, tag="rbc")
            nc.tensor.matmul(ps_rbc[:], onesr[:], recip[:], start=True, stop=True)
            # y[c, (ch,b)] = sum_h W[h, c] * xa[h, b], c split into halves on
            # partitions, h accumulated over k-halves.
            ps_y = ps.tile([128, 2 * BL], dt, tag="y")
            for ch in range(2):
                for kh in range(2):
                    nc.tensor.matmul(
                        ps_y[:, ch * BL : (ch + 1) * BL],
                        w_sb[:, kh * C + ch * 128 : kh * C + ch * 128 + 128],
                        sb_xa[kh][:],
                        start=(kh == 0),
                        stop=(kh == 1),
                    )

            # Funnel y and the recip broadcast into SBUF via DVE (PSUM-direct
            # DVE reads measured ~2x slower, and fills need single-producer
            # operands). sb_rbc first: its producer matmul retired earlier.
            # Fold normalize+relu into one tiny (128, 4) op so the big fills
            # are pure fp16 broadcast copies (2x DVE fast path): sb_yn[c, j]
            # = max(y[c, j] * recip[b], 0) in fp16.
            sb_rbc = sb.tile([128, BL], dt)
            nc.vector.tensor_copy(out=sb_rbc[:], in_=ps_rbc[:])
            sb_y = sb.tile([128, 2 * BL], mybir.dt.float16)
            nc.vector.tensor_copy(out=sb_y[:], in_=ps_y[:])
            sb_yn = sb.tile([128, 2 * BL], mybir.dt.float16)
            for ch in range(2):
                for b in range(BL):
                    j = ch * BL + b
                    nc.vector.tensor_scalar(
                        out=sb_yn[:, j : j + 1],
                        in0=sb_y[:, j : j + 1],
                        scalar1=sb_rbc[:, b : b + 1],
                        scalar2=0.0,
                        op0=alu.mult,
                        op1=alu.max,
                    )

            # Broadcast fills + chunked stores. Each fill is one DVE
            # tensor_scalar: max(y[c, j] * recip[b], 0) broadcast over the
            # chunk's pixels; each store is a plain 2D DMA of 128 c-lines.
            # Triggers MUST come from the ACT engine: its queue is the fast
            # hardware-dynamic one (GpSimd's queue is software-assisted and
            # measured ~20% lower stream bandwidth with multi-us triggers).
            segs = []
            lo = 0
            for w_seg in RAMP:
                segs.append((0, 0, lo, w_seg))
                lo += w_seg
            segs += [(0, 1, 0, P), (1, 0, 0, P), (1, 1, 0, P)]

            bc = {}
            for b in range(BL):
                for ch in range(2):
                    bc[(b, ch)] = sb.tile(
                        [128, P], mybir.dt.float16, tag=f"bc{b}{ch}", name=f"bc{b}{ch}"
                    )
            for b, ch, lo, w_seg in segs:
                j = ch * BL + b
                t = bc[(b, ch)]
                nc.vector.tensor_copy(
                    out=t[:, lo : lo + w_seg],
                    in_=sb_yn[:, j : j + 1].to_broadcast([128, w_seg]),
                )
                nc.scalar.dma_start(
                    out=out_d[b][ch * 128 : (ch + 1) * 128, lo : lo + w_seg],
                    in_=t[:, lo : lo + w_seg],
                )
    _fix_tail_drain(nc)
    return nc


def _fix_tail_drain(nc):
    """Walrus in this toolchain accepts very few sync waits per instruction.
    Two post-passes, both relying on the fact that all 9 DMAs here run
    through the ACT engine's single FIFO HW queue (completion order = issue
    order), and every instruction is strictly upstream of the last
    output-chunk DMA:

    1. Tile rotates DMA completions over 8 DMAHW sems; the 9th dma_start
       reuses the first input DMA's sem and its trigger gains a second
       sync-wait (queue-slot reuse guard) on top of its DVE fill wait. That
       guard is implied by the fill (the fill is transitively downstream of
       the input DMA), so drop the DMAHW wait and keep the DVE wait.
    2. Tile's kernel-tail drain waits on every semaphore; waiting on the
       final output chunk's completion sem alone is sufficient."""
    import bass_rust

    out_sems = []
    for ins in nc.inst_map.values():
        si = ins.sync_info
        if type(ins).__name__ != "InstDMACopy" or si is None:
            continue
        if len(si.on_wait) > 1:
            keep = [w for w in si.on_wait if not w.ant_name.startswith("DMAHW")]
            assert len(keep) == 1, [w.ant_name for w in si.on_wait]
            ins.sync_info = bass_rust.SyncInfo(
                on_wait=keep, on_update=list(si.on_update)
            )
        if "out_set" in str(ins) and len(si.on_update) > 0:
            out_sems.append(si.on_update[0].ant_name)
    assert out_sems, "output DMA completion sems not found"
    # All output chunks share the ACT engine's FIFO HW queue, so the last
    # chunk's completion sem covers every output transfer; the input DMA
    # (sync queue) is transitively upstream of every fill. The single
    # master tail drain keeps just that one wait.
    out_sem = out_sems[-1]
    for ins in nc.inst_map.values():
        si = ins.sync_info
        if type(ins).__name__ == "InstDrain" and si is not None and len(si.on_wait) > 1:
            keep = [w for w in si.on_wait if w.ant_name == out_sem]
            assert len(keep) == 1, (out_sem, [w.ant_name for w in si.on_wait])
            ins.sync_info = bass_rust.SyncInfo(
                on_wait=keep, on_update=list(si.on_update)
            )


def _get_nc():
    if "nc" not in _cache:
        _cache["nc"] = _build_nc()
    return _cache["nc"]


def _pack_cin(x_shard, nfh, w):
    """Pack one core's inputs into the (128, NCOLS) tensor."""
    cin = np.zeros((128, NCOLS), dtype=np.float32)
    cin[:, COL_W : COL_W + C] = w[0:128, :]
    cin[:, COL_W + C : COL_W + 2 * C] = w[128:256, :]
    xr = x_shard.reshape(BN, HID)
    xT = np.ascontiguousarray(xr.T)  # (256, 14)
    cin[:, COL_XT : COL_XT + BN] = xT[0:128]
    cin[:, COL_XT + BN : COL_XT + 2 * BN] = xT[128:256]
    cin[:, COL_NFH] = nfh[0:128, 0]
    cin[:, COL_NFH + 1] = nfh[128:256, 0]
    cin[0:BN, COL_XR : COL_XR + HID] = xr
    for b in range(BL):
        cin[b * NODES : (b + 1) * NODES, COL_BM + b] = 1.0
    return cin


def _make_in_maps(input, node_fea_for_hidden, weight):
    x_full = np.asarray(input, dtype=np.float32)[0]  # (B, N, HID)
    nfh = np.asarray(node_fea_for_hidden, dtype=np.float32)
    w = np.asarray(weight, dtype=np.float32)
    return [
        {"cin": _pack_cin(x_full[i * BL : (i + 1) * BL], nfh, w)}
        for i in range(NCORES)
    ]


def _run(in_maps, trace=False, **kwargs):
    nc = _get_nc()
    return run_bass_kernel_spmd(nc, in_maps, list(range(NCORES)), trace=trace, **kwargs)


def kernel(input, res_feature, node_fea_for_res, node_fea_for_hidden, weight):
    in_maps = _make_in_maps(input, node_fea_for_hidden, weight)
    res = _run(in_maps)
    shards = [res.results[i]["out"] for i in range(NCORES)]  # each (BL, C, P) fp16
    full = np.concatenate(shards, axis=0)  # (B, C, P)
    return full.reshape(B, C, H, W).astype(np.float32)
